# revision 1
# baseline (speedup 1.0000x reference)
"""Causal STFT kernel for Trainium2 (8 NeuronCores, data-parallel over batch).

Problem: x [16, 524288] f32 -> mag [16, 513, 2048] f32.
  Per batch: causal pad 1023 zeros on the left, frames of 1024 at hop 256
  (2048 frames), multiply by Hann-windowed DFT basis (1026 x 1024), take
  per-bin magnitude sqrt(clip(re^2 + im^2, 1e-12)).

Sharding: batch dim split 2 per core across 8 cores (SPMD, no collectives).

Default device strategy, MODE='fold' (~91 us HW time, ~2.6e-4 rel err):
  - Host relayouts each padded signal (pure layout, each element stored once
    per view): C_h[p, c] = xp[256c + 128h + p] and a partition-reversed copy
    D_g[p, c] = xp[256c - 128g - p], both fp16.  Frame t window position
    m = 128a + p is then C_{a&1}[p, t + (a>>1)], and position 1024 - m is
    D_{a&1}[p, t + 4 - (a>>1)].
  - The Hann-windowed DFT rows are symmetric (cos) / antisymmetric (sin)
    about the frame center, so DVE folds Fplus = C + D, Fminus = C - D
    halve the tensor-engine contraction to K = 512.  The m=0 fold slot has
    zero window weight and is repurposed for the self-paired center sample
    x[512] whose weight column is w2[:, 512]; this also packs the bin-512
    cos row in as a 513th M column of the cos weights.
  - TensorE (fp16): per (batch, 512-frame tile): 4 cos 128-bin tiles +
    1-row bin-512 tile + 4 sin tiles, each accumulating 4 K-chunks in PSUM.
  - ACT squares PSUM pairs, DVE adds them with the eps clip, ACT takes the
    sqrt, outputs stream out per 128x512 tile.  Bin 0 falls out of the
    all-zero sin_0 row; bin 512 is |re_512| via ACT Abs.
"""

import os
import sys

import numpy as np

for _p in ("/opt/trn_rl_repo",):
    if _p not in sys.path and os.path.isdir(_p):
        sys.path.insert(0, _p)

N_FFT = 1024
HOP = 256
CACHE = N_FFT - 1  # 1023 zeros of causal left pad
BATCH = 16
SAMPLES = HOP * 2048
L = 2048  # frames per batch
F = 513  # output bins per batch
NCORES = 8
BPC = BATCH // NCORES  # batches per core = 2
NCHUNK = (CACHE + SAMPLES + 1) // HOP  # 2052 chunks of 256 after padding
KT = N_FFT // 128  # 8 contraction tiles
NT = L // 512  # 4 frame tiles
QT = 4  # 4 (re, im) pair tiles of 128 bins

# matmul mode: 'fold' (K=512 via window symmetry, fp16), or direct K=1024
# modes 'f32r' (full-rate fp32-in), 'f16', 'bf16', 'f32' (4x slow)
MODE = os.environ.get("STFT_MM_DTYPE", "fold")

_PROGRAM_CACHE = {}


def _mm_dtype(mybir):
    return {
        "f32r": mybir.dt.float32r,
        "f32": mybir.dt.float32,
        "f16": mybir.dt.float16,
        "bf16": mybir.dt.bfloat16,
    }[MODE]


def _np_w_dtype():
    import ml_dtypes

    return {
        "f32r": np.float32,
        "f32": np.float32,
        "f16": np.float16,
        "bf16": ml_dtypes.bfloat16,
    }[MODE]


def _build_program():
    import concourse.bacc as bacc
    import concourse.mybir as mybir
    import concourse.tile as tile

    DT = _mm_dtype(mybir)
    f32 = mybir.dt.float32
    needs_cast = MODE in ("f16", "bf16")

    nc = bacc.Bacc("TRN2", target_bir_lowering=False, debug=False)
    w_in = nc.declare_dram_parameter("w", [KT, 128, 1024], DT, isOutput=False)
    c_in = nc.declare_dram_parameter(
        "c", [BPC, 2, 128, NCHUNK], f32 if needs_cast else DT, isOutput=False
    )
    out = nc.declare_dram_parameter("out", [BPC, F, L], f32, isOutput=True)

    # column chunks for the signal loads: n-tile j only needs cols
    # [512j, 512j+516), so chunked DMA+cast lets matmuls start early.
    CB = [0, 516, 1032, 1548, NCHUNK]

    with tile.TileContext(nc) as tc:
        with (
            tc.tile_pool(name="wp", bufs=1) as wp,
            tc.tile_pool(name="cp", bufs=1) as cp,
            tc.tile_pool(name="castp", bufs=1) as castp,
            tc.tile_pool(name="ps", bufs=3, space="PSUM") as ps,
            tc.tile_pool(name="sqp", bufs=3) as sqp,
            tc.tile_pool(name="sp", bufs=3) as sp,
            tc.tile_pool(name="stp", bufs=3) as stp,
            tc.tile_pool(name="r512p", bufs=2) as r512p,
        ):
            w_sb = [None] * KT

            def load_w(k):
                wt = wp.tile([128, 1024], DT, name=f"w{k}")
                nc.sync.dma_start(wt[:], w_in[k])
                w_sb[k] = wt

            c_sb = [[None, None] for _ in range(BPC)]

            def load_c(b, chunks):
                for h in range(2):
                    if c_sb[b][h] is None:
                        c_sb[b][h] = cp.tile(
                            [128, NCHUNK], f32 if needs_cast else DT, name=f"c{b}{h}"
                        )
                        if needs_cast:
                            cast = castp.tile([128, NCHUNK], DT, name=f"cc{b}{h}")
                            c_sb[b][h] = (c_sb[b][h], cast)
                for j in chunks:
                    lo, hi = CB[j], CB[j + 1]
                    for h in range(2):
                        t = c_sb[b][h]
                        if needs_cast:
                            raw, cast = t
                            nc.sync.dma_start(raw[:, lo:hi], c_in[b, h, :, lo:hi])
                            nc.vector.tensor_copy(cast[:, lo:hi], raw[:, lo:hi])
                        else:
                            nc.sync.dma_start(t[:, lo:hi], c_in[b, h, :, lo:hi])

            def c_tile(b, h):
                t = c_sb[b][h]
                return t[1] if needs_cast else t

            # order: w0 + first chunk of batch 0 first so the PE can start,
            # then the rest of the weights, then remaining signal chunks.
            load_w(0)
            load_c(0, [0])
            for k in range(1, KT):
                load_w(k)
            load_c(0, [1, 2, 3])
            load_c(1, [0, 1, 2, 3])

            def rhs(b, k, n):
                off = n * 512 + (k >> 1)
                return c_tile(b, k & 1)[:, off : off + 512]

            for b in range(BPC):
                for n in range(NT):
                    for q in range(QT):
                        ps_re = ps.tile([128, 512], f32, name=f"psre{b}{n}{q}", tag="psre")
                        ps_im = ps.tile([128, 512], f32, name=f"psim{b}{n}{q}", tag="psim")
                        for k in range(KT):
                            nc.tensor.matmul(
                                ps_re[:],
                                w_sb[k][:, q * 128 : (q + 1) * 128],
                                rhs(b, k, n),
                                start=(k == 0),
                                stop=(k == KT - 1),
                            )
                        for k in range(KT):
                            nc.tensor.matmul(
                                ps_im[:],
                                w_sb[k][:, (q + 4) * 128 : (q + 5) * 128],
                                rhs(b, k, n),
                                start=(k == 0),
                                stop=(k == KT - 1),
                            )
                        sq_re = sqp.tile([128, 512], f32, name=f"sqre{b}{n}{q}", tag="sqre")
                        sq_im = sqp.tile([128, 512], f32, name=f"sqim{b}{n}{q}", tag="sqim")
                        nc.scalar.square(sq_re[:], ps_re[:])
                        nc.scalar.square(sq_im[:], ps_im[:])
                        s = sp.tile([128, 512], f32, name=f"s{b}{n}{q}", tag="s")
                        # s = max(re^2, eps) + im^2  (~= clip(re^2+im^2, eps),
                        # exact whenever re^2+im^2 >= eps)
                        nc.vector.scalar_tensor_tensor(
                            s[:],
                            sq_re[:],
                            1e-12,
                            sq_im[:],
                            op0=mybir.AluOpType.max,
                            op1=mybir.AluOpType.add,
                        )
                        if q == 0:
                            # tile pair 0/4 packs cos_512 into the im slot of
                            # row 0; bin 0 is |re_0| and bin 512 is |re_512|.
                            nc.vector.tensor_scalar_max(s[0:1, :], sq_re[0:1, :], 1e-12)
                            r512 = r512p.tile([1, 512], f32, name=f"r512{b}{n}", tag="r512")
                            nc.vector.tensor_scalar_max(r512[:], sq_im[0:1, :], 1e-12)
                            nc.scalar.sqrt(r512[:], r512[:])
                            nc.gpsimd.dma_start(
                                out[b, F - 1 : F, n * 512 : (n + 1) * 512], r512[:]
                            )
                        st = stp.tile([128, 512], f32, name=f"st{b}{n}{q}", tag="st")
                        nc.scalar.sqrt(st[:], s[:])
                        nc.sync.dma_start(
                            out[b, q * 128 : (q + 1) * 128, n * 512 : (n + 1) * 512],
                            st[:],
                        )
    nc.finalize()
    return nc


def _build_program_fold():
    """K=512 variant: the Hann-windowed DFT rows are (anti)symmetric about
    the frame center, so contracting folded frames

      Fplus[m]  = x[m] + x[1024-m]   (cos rows,  m = 1..511)
      Fminus[m] = x[m] - x[1024-m]   (sin rows)

    halves the tensor-engine work.  Slot m=0 carries zero window weight and
    is repurposed for the self-paired center sample x[512] (weight column
    w2[:, 512]), which also folds bin 512 in as one extra M row.  Folds are
    cheap shifted-slice adds of the C layout and a host-built partition-
    reversed copy D_g[p, c] = xp[256c - 128g - p].
    """
    import concourse.bacc as bacc
    import concourse.mybir as mybir
    import concourse.tile as tile

    f32 = mybir.dt.float32
    f16 = mybir.dt.float16

    nc = bacc.Bacc("TRN2", target_bir_lowering=False, debug=False)
    wp_in = nc.declare_dram_parameter("wp", [4, 128, 513], f16, isOutput=False)
    wm_in = nc.declare_dram_parameter("wm", [4, 128, 512], f16, isOutput=False)
    c_in = nc.declare_dram_parameter("c", [BPC, 2, 128, NCHUNK], f16, isOutput=False)
    d_in = nc.declare_dram_parameter("d", [BPC, 2, 128, NCHUNK], f16, isOutput=False)
    out = nc.declare_dram_parameter("out", [BPC, F, L], f32, isOutput=True)

    CH0 = 516  # first-column chunk so the pipeline can start early

    with tile.TileContext(nc) as tc:
        with (
            tc.tile_pool(name="wtp", bufs=1) as wtp,
            tc.tile_pool(name="cdp", bufs=2) as cdp,
            tc.tile_pool(name="fp", bufs=2) as fp,
            tc.tile_pool(name="pcp", bufs=4, space="PSUM") as pcp,
            tc.tile_pool(name="psp", bufs=3, space="PSUM") as psp,
            tc.tile_pool(name="p512p", bufs=1, space="PSUM") as p512p,
            tc.tile_pool(name="sqp", bufs=3) as sqp,
            tc.tile_pool(name="sp", bufs=3) as sp,
            tc.tile_pool(name="stp", bufs=3) as stp,
            tc.tile_pool(name="r512p", bufs=2) as r512p,
        ):
            cd_sb = [None] * BPC

            def load_cd(b, lo, hi):
                if cd_sb[b] is None:
                    cd_sb[b] = (
                        [
                            cdp.tile([128, NCHUNK], f16, name=f"c{h}", tag=f"c{h}")
                            for h in range(2)
                        ],
                        [
                            cdp.tile([128, NCHUNK], f16, name=f"d{h}", tag=f"d{h}")
                            for h in range(2)
                        ],
                    )
                c_sb, d_sb = cd_sb[b]
                for h in range(2):
                    nc.sync.dma_start(c_sb[h][:, lo:hi], c_in[b, h, :, lo:hi])
                    nc.sync.dma_start(d_sb[h][:, lo:hi], d_in[b, h, :, lo:hi])

            # DMA ring order: batch-0 first chunk, cos weights, batch-0 rest,
            # sin weights — matches the order the PE consumes them.
            load_cd(0, 0, CH0)

            wp_sb, wm_sb = [], []
            for a in range(4):
                t = wtp.tile([128, 513], f16, name=f"wpa{a}")
                nc.sync.dma_start(t[:], wp_in[a])
                wp_sb.append(t)

            load_cd(0, CH0, NCHUNK)

            for a in range(4):
                t = wtp.tile([128, 512], f16, name=f"wma{a}")
                nc.sync.dma_start(t[:], wm_in[a])
                wm_sb.append(t)

            for b in range(BPC):
                c_sb, d_sb = cd_sb[b]
                fpl = [
                    fp.tile([128, L], f16, name=f"fp{a}", tag=f"fp{a}")
                    for a in range(4)
                ]
                fmi = [
                    fp.tile([128, L], f16, name=f"fm{a}", tag=f"fm{a}")
                    for a in range(4)
                ]

                def fold_cols(lo, hi):
                    # plus folds first (cos matmuls consume them first)
                    for sign in range(2):
                        for a in range(4):
                            g = a & 1
                            ao = a >> 1
                            cs = c_sb[g][:, lo + ao : hi + ao]
                            ds = d_sb[g][:, lo + 4 - ao : hi + 4 - ao]
                            nc.vector.tensor_tensor(
                                (fpl if sign == 0 else fmi)[a][:, lo:hi],
                                cs,
                                ds,
                                op=mybir.AluOpType.add
                                if sign == 0
                                else mybir.AluOpType.subtract,
                            )
                        # slot m=0 of both folds carries the self-paired center
                        # sample x[512]; its weight column is w2[:, 512], which
                        # is nonzero even for sin rows (f32 rounding of the
                        # reference angle leaves ~1e-4 there).
                        nc.vector.tensor_copy(
                            (fpl if sign == 0 else fmi)[0][0:1, lo:hi],
                            c_sb[0][0:1, lo + 2 : hi + 2],
                        )

                fold_cols(0, 512)
                fold_cols(512, L)

                for n in range(NT):
                    nsl = slice(n * 512, (n + 1) * 512)
                    if b + 1 < BPC and n == 0:
                        # batch-1 signal streams in while batch-0 computes
                        load_cd(b + 1, 0, CH0)
                        load_cd(b + 1, CH0, NCHUNK)
                    # interleave cos/sin per q so each pair's magnitude
                    # pipeline starts as early as possible
                    pc_t, ps_t = [], []
                    for q in range(QT):
                        pc = pcp.tile([128, 512], f32, name=f"pc{b}{n}{q}", tag="pc")
                        for a in range(4):
                            nc.tensor.matmul(
                                pc[:],
                                wp_sb[a][:, q * 128 : (q + 1) * 128],
                                fpl[a][:, nsl],
                                start=(a == 0),
                                stop=(a == 3),
                            )
                        pc_t.append(pc)
                        pss = psp.tile([128, 512], f32, name=f"psn{b}{n}{q}", tag="ps")
                        for a in range(4):
                            nc.tensor.matmul(
                                pss[:],
                                wm_sb[a][:, q * 128 : (q + 1) * 128],
                                fmi[a][:, nsl],
                                start=(a == 0),
                                stop=(a == 3),
                            )
                        ps_t.append(pss)
                    p512 = p512p.tile([1, 512], f32, name=f"p512{b}{n}", tag="p512")
                    for a in range(4):
                        nc.tensor.matmul(
                            p512[:],
                            wp_sb[a][:, 512:513],
                            fpl[a][:, nsl],
                            start=(a == 0),
                            stop=(a == 3),
                        )

                    r512 = r512p.tile([1, 512], f32, name=f"r512{b}{n}", tag="r512")
                    nc.scalar.activation(
                        r512[:], p512[:], mybir.ActivationFunctionType.Abs
                    )
                    nc.vector.tensor_scalar_max(r512[:], r512[:], 1e-6)
                    nc.gpsimd.dma_start(out[b, F - 1 : F, nsl], r512[:])

                    for q in range(QT):
                        sq_c = sqp.tile([128, 512], f32, name=f"sqc{b}{n}{q}", tag="sqc")
                        sq_s = sqp.tile([128, 512], f32, name=f"sqs{b}{n}{q}", tag="sqs")
                        if q == 3 and not (b == BPC - 1 and n == NT - 1):
                            # relieve the saturated ACT: square the last pair
                            # on DVE via fp16 PSUM copies (fp16 TT runs 2x)
                            cp_c = sqp.tile(
                                [128, 512], f16, name=f"cpc{b}{n}{q}", tag="cpc"
                            )
                            cp_s = sqp.tile(
                                [128, 512], f16, name=f"cps{b}{n}{q}", tag="cps"
                            )
                            nc.vector.tensor_copy(cp_c[:], pc_t[q][:])
                            nc.vector.tensor_copy(cp_s[:], ps_t[q][:])
                            nc.vector.tensor_tensor(
                                sq_c[:], cp_c[:], cp_c[:], op=mybir.AluOpType.mult
                            )
                            nc.vector.tensor_tensor(
                                sq_s[:], cp_s[:], cp_s[:], op=mybir.AluOpType.mult
                            )
                        else:
                            nc.scalar.square(sq_c[:], pc_t[q][:])
                            nc.scalar.square(sq_s[:], ps_t[q][:])
                        s = sp.tile([128, 512], f32, name=f"s{b}{n}{q}", tag="s")
                        # sin bin-0 row is zero, so row 0 automatically gives
                        # sqrt(max(re0^2, eps)) = mag of bin 0.
                        nc.vector.scalar_tensor_tensor(
                            s[:],
                            sq_c[:],
                            1e-12,
                            sq_s[:],
                            op0=mybir.AluOpType.max,
                            op1=mybir.AluOpType.add,
                        )
                        st = stp.tile([128, 512], f32, name=f"st{b}{n}{q}", tag="st")
                        nc.scalar.sqrt(st[:], s[:])
                        nc.sync.dma_start(out[b, q * 128 : (q + 1) * 128, nsl], st[:])
    nc.finalize()
    return nc


def _get_program():
    key = MODE
    if key not in _PROGRAM_CACHE:
        _PROGRAM_CACHE[key] = (
            _build_program_fold() if MODE == "fold" else _build_program()
        )
    return _PROGRAM_CACHE[key]


def _make_weight_np():
    n = np.arange(N_FFT, dtype=np.float32)
    k = np.arange(N_FFT // 2 + 1, dtype=np.float32)[:, None]
    ang = (-2.0 * np.pi / N_FFT) * k * n[None, :]
    win = 0.5 * (1.0 - np.cos(2.0 * np.pi * n / N_FFT))
    return np.concatenate([np.cos(ang), np.sin(ang)], axis=0) * win  # [1026, 1024]


def _pack_weight(weight):
    if weight is None:
        w2 = _make_weight_np()
    else:
        w2 = np.asarray(weight, dtype=np.float32).reshape(2 * (N_FFT // 2 + 1), N_FFT)
    # rows: cos 0..511, cos 512, sin 1..511  (sin 0 and sin 512 are zero rows)
    w_eff = np.concatenate([w2[0:512], w2[512:513], w2[514:1025]], axis=0)
    # Wt[k, p, m] = w_eff[m, 128k + p]
    wt = np.ascontiguousarray(
        w_eff.T.reshape(KT, 128, N_FFT), dtype=np.float32
    ).astype(_np_w_dtype())
    return wt


def _frame_layout(xb):
    """[SAMPLES] f32 -> C[2, 128, NCHUNK] with C[h, p, c] = xp[256c + 128h + p]."""
    xp = np.empty(NCHUNK * HOP, dtype=np.float32)
    xp[:CACHE] = 0.0
    xp[CACHE : CACHE + SAMPLES] = xb
    xp[CACHE + SAMPLES :] = 0.0
    return np.ascontiguousarray(xp.reshape(NCHUNK, 2, 128).transpose(1, 2, 0))


def _frame_layout_rev(xb):
    """Partition-reversed copy: D[g, p, c] = xp[256c - 128g - p] (0 if oob)."""
    xp = np.empty(NCHUNK * HOP, dtype=np.float32)
    xp[:CACHE] = 0.0
    xp[CACHE : CACHE + SAMPLES] = xb
    xp[CACHE + SAMPLES :] = 0.0
    c = 256 * np.arange(NCHUNK, dtype=np.int64)[None, None, :]
    g = 128 * np.arange(2, dtype=np.int64)[:, None, None]
    p = np.arange(128, dtype=np.int64)[None, :, None]
    idx = c - g - p
    d = xp[np.clip(idx, 0, None)]
    d[idx < 0] = 0.0
    return np.ascontiguousarray(d)


def _pack_weight_fold(weight):
    if weight is None:
        w2 = _make_weight_np()
    else:
        w2 = np.asarray(weight, dtype=np.float32).reshape(2 * (N_FFT // 2 + 1), N_FFT)
    # fold column j contracts x[j] + x[1024-j] (j = 1..511); slot j=0 carries
    # the center sample x[512], whose weight column is w2[:, 512].
    colmap = np.concatenate([[512], np.arange(1, 512)])
    wplus = w2[0:513][:, colmap]  # cos bins 0..512  [513, 512]
    wminus = w2[513:1025][:, colmap]  # sin bins 0..511 (row 0 zero)  [512, 512]
    wp = np.ascontiguousarray(wplus.T.reshape(4, 128, 513)).astype(np.float16)
    wm = np.ascontiguousarray(wminus.T.reshape(4, 128, 512)).astype(np.float16)
    return wp, wm


def _in_maps(x, weight):
    if MODE == "fold":
        wp, wm = _pack_weight_fold(weight)
        maps = []
        for i in range(NCORES):
            c = np.stack([_frame_layout(x[BPC * i + b]) for b in range(BPC)])
            d = np.stack([_frame_layout_rev(x[BPC * i + b]) for b in range(BPC)])
            maps.append(
                {"wp": wp, "wm": wm, "c": c.astype(np.float16), "d": d.astype(np.float16)}
            )
        return maps
    wt = _pack_weight(weight)
    maps = []
    for i in range(NCORES):
        c = np.stack([_frame_layout(x[BPC * i + b]) for b in range(BPC)])
        maps.append({"w": wt, "c": c})
    return maps


def kernel(x, weight=None, **_unused):
    from concourse.bass_utils import run_bass_kernel_spmd

    x = np.asarray(x, dtype=np.float32)
    assert x.shape == (BATCH, SAMPLES), x.shape

    nc = _get_program()
    res = run_bass_kernel_spmd(nc, _in_maps(x, weight), core_ids=list(range(NCORES)))

    out = np.empty((BATCH, F, L), dtype=np.float32)
    for i in range(NCORES):
        out[BPC * i : BPC * (i + 1)] = res.results[i]["out"]
    return out

